# revision 6
# baseline (speedup 1.0000x reference)
"""Trainium2 Bass kernel for nn_Attention_87737591923407 (PVT-style spatial-
reduction attention with LoRA on q/v).

Sharding: 8 cores = 2 batches x 4 sequence chunks (2304 rows each). The
spatial-reduction conv is sharded over c_in within each batch group and
combined with an AllReduce; everything else is row-parallel.

Self-contained: only imports concourse (installed site package) + numpy.
"""
import numpy as np

import concourse.bass as bass
import concourse.mybir as mybir
import concourse.tile as tile
from concourse import bacc
from concourse import bass_utils

# Problem constants (hardcoded per contract)
B, N, C = 2, 9216, 512
HEAD, SR, R = 8, 4, 32
D = C // HEAD                  # 64
NKV = (96 // SR) * (96 // SR)  # 576
SCALING = 4.0 / 32.0
EPS = 1e-5
SM_SCALE = float(D) ** -0.5    # 0.125

N_CORES = 8
NCHUNK = N // 4            # 2304 rows per core
NF = 256                   # q-rows per inner chunk
NCH = NCHUNK // NF         # 9 inner chunks
CIN_SL = 16 * C // 4       # 2048 contraction rows of conv per core
MPAD = 640                 # padded kv length (5 x 128)

F32 = mybir.dt.float32
F32R = mybir.dt.float32r
Exp = mybir.ActivationFunctionType.Exp
Ln = mybir.ActivationFunctionType.Ln
Copy = mybir.ActivationFunctionType.Copy
ADD = mybir.AluOpType.add
SUB = mybir.AluOpType.subtract
MULT = mybir.AluOpType.mult


def build_kernel():
    nc = bacc.Bacc("TRN2", target_bir_lowering=False, debug=False,
                   num_devices=N_CORES)

    # ---- per-core DRAM inputs ----
    def din(name, shape):
        return nc.dram_tensor(name, shape, F32, kind="ExternalInput")

    xT = din("xT", [C, NCHUNK])          # x[b, chunk].T
    patchT = din("patchT", [CIN_SL, NKV])
    wsrT = din("wsrT", [CIN_SL, C])
    wqT = din("wqT", [C, C])
    wkT = din("wkT", [C, C])             # LN-gamma folded
    wvT = din("wvT", [C, C])             # LN-gamma folded
    wpT = din("wpT", [C, C])
    aqT = din("aqT", [C, R])
    bqT = din("bqT", [R, C])             # * SCALING
    avT = din("avT", [C, R])             # LN-gamma folded
    bvT = din("bvT", [R, C])             # * SCALING
    b_q = din("b_q", [1, C])
    b_k = din("b_k", [1, C])             # + w_k @ ln_b
    b_v = din("b_v", [1, C])             # + w_v @ ln_b
    b_sr4 = din("b_sr4", [1, C])         # b_sr / 4
    b_p = din("b_p", [1, C])
    avb = din("avb", [1, R])             # A_v_eff @ ln_b

    out_d = nc.dram_tensor("out", [NCHUNK, C], F32, kind="ExternalOutput")

    def chunked(ap):
        return ap.rearrange("(o p) n -> p o n", p=128)

    with tile.TileContext(nc) as tc:
        with (
            tc.tile_pool(name="const", bufs=1) as cp,
            tc.tile_pool(name="big", bufs=1) as bp,
            tc.tile_pool(name="psA", bufs=1, space="PSUM") as psA,
            tc.tile_pool(name="psST", bufs=1, space="PSUM") as psST,
            tc.tile_pool(name="psAV", bufs=1, space="PSUM") as psAV,
            tc.tile_pool(name="dram", bufs=1, space="DRAM") as dp,
        ):
            # ---------------- load weights / constants ----------------
            xT_ch = chunked(xT.ap())

            wq_sb = cp.tile([128, 4, C], F32R)
            nc.gpsimd.dma_start(wq_sb[:], chunked(wqT.ap()))
            wk_sb = cp.tile([128, 4, C], F32R)
            nc.gpsimd.dma_start(wk_sb[:], chunked(wkT.ap()))
            wv_sb = cp.tile([128, 4, C], F32R)
            nc.gpsimd.dma_start(wv_sb[:], chunked(wvT.ap()))
            wp_sb = cp.tile([128, 4, C], F32R)
            nc.gpsimd.dma_start(wp_sb[:], chunked(wpT.ap()))
            aq_sb = cp.tile([128, 4, R], F32R)
            nc.gpsimd.dma_start(aq_sb[:], chunked(aqT.ap()))
            av_sb = cp.tile([128, 4, R], F32R)
            nc.gpsimd.dma_start(av_sb[:], chunked(avT.ap()))
            bq_sb = cp.tile([R, C], F32R)
            nc.gpsimd.dma_start(bq_sb[:], bqT.ap())
            bv_sb = cp.tile([R, C], F32R)
            nc.gpsimd.dma_start(bv_sb[:], bvT.ap())

            bias_q = cp.tile([1, C], F32R)
            nc.gpsimd.dma_start(bias_q[:], b_q.ap())
            bias_k = cp.tile([1, C], F32R)
            nc.gpsimd.dma_start(bias_k[:], b_k.ap())
            bias_v = cp.tile([1, C], F32R)
            nc.gpsimd.dma_start(bias_v[:], b_v.ap())
            bias_sr = cp.tile([1, C], F32R)
            nc.gpsimd.dma_start(bias_sr[:], b_sr4.ap())
            bias_p = cp.tile([1, C], F32R)
            nc.gpsimd.dma_start(bias_p[:], b_p.ap())
            bias_av = cp.tile([1, R], F32R)
            nc.gpsimd.dma_start(bias_av[:], avb.ap())

            ones_f = cp.tile([1, 512], F32)
            nc.any.memset(ones_f[:], 1.0)
            ones_r = cp.tile([1, 512], F32R)
            nc.vector.tensor_copy(ones_r[:], ones_f[:])
            onesc = cp.tile([128, 1], F32)
            nc.any.memset(onesc[:], 1.0)

            z_sb = bp.tile([128, 4, NKV], F32R)
            kT_sb = bp.tile([128, 4, MPAD], F32R)
            v_sb = bp.tile([128, 5, HEAD, D + 1], F32R)

            with tc.tile_pool(name="mid", bufs=1) as mp:
                xs_part = mp.tile([128, 4, NKV], F32, tag="xsbuf")
                with tc.tile_pool(name="convp", bufs=1) as vp:
                    pt_sb = vp.tile([128, 16, NKV], F32R)
                    nc.gpsimd.dma_start(pt_sb[:], chunked(patchT.ap()))
                    wsr_sb = vp.tile([128, 16, C], F32R)
                    nc.gpsimd.dma_start(wsr_sb[:], chunked(wsrT.ap()))

                    for M in range(4):
                        for nh in range(2):
                            pc = psA.tile([128, 512], F32, tag="psa",
                                          name=f"conv_{M}_{nh}")
                            nsl = slice(288 * nh, 288 * nh + 288)
                            for K in range(16):
                                nc.tensor.matmul(
                                    pc[:, :288],
                                    wsr_sb[:, K, 128 * M:128 * M + 128],
                                    pt_sb[:, K, nsl],
                                    start=(K == 0), stop=False)
                            nc.tensor.matmul(
                                pc[:, :288],
                                bias_sr[:, 128 * M:128 * M + 128],
                                ones_r[:, :288], start=False, stop=True)
                            nc.scalar.copy(xs_part[:, M, nsl], pc[:, :288])

                # ---------------- AllReduce over batch group ----------------
                cc_in = dp.tile([4, 128, NKV], F32)
                cc_out = dp.tile([4, 128, NKV], F32)
                nc.sync.dma_start(cc_in[:].rearrange("o p n -> p o n"), xs_part[:])
                nc.gpsimd.collective_compute(
                    "AllReduce", ADD,
                    replica_groups=[[0, 1, 2, 3], [4, 5, 6, 7]],
                    ins=[cc_in[:].opt()],
                    outs=[cc_out[:].opt()],
                )
                xs_g = mp.tile([128, 4, NKV], F32, tag="xsbuf", name="xs_g")
                nc.sync.dma_start(xs_g[:], cc_out[:].rearrange("o p n -> p o n"))

                # ---------------- LayerNorm stats ----------------
                xs_sq = mp.tile([128, 4, NKV], F32, tag="scr", name="xs_sq")
                nc.vector.tensor_tensor(xs_sq[:], xs_g[:], xs_g[:], MULT)
                mu = cp.tile([1, NKV], F32)
                st_ps = psA.tile([1, 512], F32, tag="psa", name="st_sum")
                for nh in range(2):
                    nsl = slice(288 * nh, 288 * nh + 288)
                    for K in range(4):
                        nc.tensor.matmul(st_ps[:, nsl if nh == 0 else slice(0, 288)],
                                         onesc[:], xs_g[:, K, nsl],
                                         start=(K == 0), stop=(K == 3))
                    nc.scalar.activation(mu[:, nsl], st_ps[:, nsl if nh == 0
                                                           else slice(0, 288)],
                                         Copy, scale=1.0 / C)
                sq = cp.tile([1, NKV], F32)
                st_ps2 = psA.tile([1, 512], F32, tag="psa", name="st_sum2")
                for nh in range(2):
                    nsl = slice(288 * nh, 288 * nh + 288)
                    for K in range(4):
                        nc.tensor.matmul(st_ps2[:, nsl if nh == 0 else slice(0, 288)],
                                         onesc[:], xs_sq[:, K, nsl],
                                         start=(K == 0), stop=(K == 3))
                    nc.scalar.activation(sq[:, nsl], st_ps2[:, nsl if nh == 0
                                                            else slice(0, 288)],
                                         Copy, scale=1.0 / C)
                # var = sq - mu^2 ; rstd = exp(-0.5*ln(var+eps))
                musq = cp.tile([1, NKV], F32)
                nc.vector.tensor_tensor(musq[:], mu[:], mu[:], MULT)
                var = cp.tile([1, NKV], F32)
                nc.vector.tensor_tensor(var[:], sq[:], musq[:], SUB)
                eps_t = cp.tile([1, 1], F32)
                nc.any.memset(eps_t[:], EPS)
                lnv = cp.tile([1, NKV], F32)
                nc.scalar.activation(lnv[:], var[:], Ln, bias=eps_t[:])
                rstd = cp.tile([1, NKV], F32)
                nc.scalar.activation(rstd[:], lnv[:], Exp, scale=-0.5)
                mub = cp.tile([128, NKV], F32)
                nc.gpsimd.partition_broadcast(mub[:], mu[:], channels=128)
                rstdb = cp.tile([128, NKV], F32)
                nc.gpsimd.partition_broadcast(rstdb[:], rstd[:], channels=128)

                # z = (xs - mu) * rstd  (LN affine folded into weights)
                z_f = mp.tile([128, 4, NKV], F32, tag="scr", name="z_f")
                nc.vector.tensor_tensor(
                    z_f[:], xs_g[:],
                    mub[:, None, :].broadcast_to((128, 4, NKV)), SUB)
                nc.vector.tensor_tensor(
                    z_sb[:], z_f[:],
                    rstdb[:, None, :].broadcast_to((128, 4, NKV)), MULT)

            # ---------------- kT (with zero pad cols) ----------------
            zpad_f = cp.tile([128, MPAD - NKV], F32)
            nc.any.memset(zpad_f[:], 0.0)
            nc.vector.tensor_copy(
                kT_sb[:, :, NKV:MPAD],
                zpad_f[:, None, :].broadcast_to((128, 4, MPAD - NKV)))
            for M in range(4):
                for nh in range(2):
                    pk = psA.tile([128, 512], F32, tag="psa", name=f"k_{M}_{nh}")
                    nsl = slice(288 * nh, 288 * nh + 288)
                    for K in range(4):
                        nc.tensor.matmul(pk[:, :288],
                                         wk_sb[:, K, 128 * M:128 * M + 128],
                                         z_sb[:, K, nsl], start=(K == 0), stop=False)
                    nc.tensor.matmul(pk[:, :288], bias_k[:, 128 * M:128 * M + 128],
                                     ones_r[:, :288], start=False, stop=True)
                    nc.scalar.copy(kT_sb[:, M, nsl], pk[:, :288])

            # ---------------- v_sb (64 dims, then ones col at D) ----------------
            vscr = cp.tile([128, D + 1], F32)
            nc.any.memset(vscr[:], 0.0)
            nc.any.memset(vscr[:, D:D + 1], 1.0)
            vzero = cp.tile([128, D + 1], F32)
            nc.any.memset(vzero[:], 0.0)
            for mc in range(4):
                nc.vector.tensor_copy(
                    v_sb[:, mc, :, :],
                    vscr[:, None, :].broadcast_to((128, HEAD, D + 1)))
            nc.vector.tensor_copy(
                v_sb[0:64, 4, :, :],
                vscr[0:64, None, :].broadcast_to((64, HEAD, D + 1)))
            nc.vector.tensor_copy(
                v_sb[64:128, 4, :, :],
                vzero[64:128, None, :].broadcast_to((64, HEAD, D + 1)))

            for mc in range(5):
                mrows = 128 if mc < 4 else 64
                pv = psA.tile([128, 512], F32, tag="psa", name=f"v_{mc}")
                for K in range(4):
                    nc.tensor.matmul(pv[:mrows, :],
                                     z_sb[:, K, 128 * mc:128 * mc + mrows],
                                     wv_sb[:, K, :], start=(K == 0), stop=False)
                nc.tensor.matmul(pv[:mrows, :], ones_r[:, :mrows], bias_v[:],
                                 start=False, stop=True)
                nc.vector.tensor_copy(v_sb[:mrows, mc, :, 0:D], pv[:mrows, :])

            # ---------------- lora-v -> lv -> permuted add into v_sb ----------
            tv_sb = cp.tile([R, NKV], F32R)
            for nh in range(2):
                ptv = psA.tile([128, 512], F32, tag="psa", name=f"tv_{nh}")
                nsl = slice(288 * nh, 288 * nh + 288)
                for K in range(4):
                    nc.tensor.matmul(ptv[:R, :288], av_sb[:, K, :], z_sb[:, K, nsl],
                                     start=(K == 0), stop=False)
                nc.tensor.matmul(ptv[:R, :288], bias_av[:], ones_r[:, :288],
                                 start=False, stop=True)
                nc.scalar.copy(tv_sb[:, nsl], ptv[:R, :288])

            lv_dram = dp.tile([NKV * C], F32)
            lv_view = lv_dram[:].rearrange("(m c) -> m c", c=C)
            with tc.tile_pool(name="lvp", bufs=2) as lp:
                for mc in range(5):
                    mrows = 128 if mc < 4 else 64
                    plv = psA.tile([128, 512], F32, tag="psa", name=f"lv_{mc}")
                    nc.tensor.matmul(plv[:mrows, :],
                                     tv_sb[:, 128 * mc:128 * mc + mrows],
                                     bv_sb[:], start=True, stop=True)
                    lv_sb = lp.tile([128, 512], F32, tag="lvsb")
                    nc.vector.tensor_copy(lv_sb[:mrows, :], plv[:mrows, :])
                    nc.sync.dma_start(lv_view[128 * mc:128 * mc + mrows, :],
                                      lv_sb[:mrows, :])
                lv3 = lv_dram[:].rearrange("(h m dd) -> h m dd",
                                           h=HEAD, m=NKV, dd=D)
                for mc in range(5):
                    mrows = 128 if mc < 4 else 64
                    zt = lp.tile([128, HEAD, D], F32, tag="zperm")
                    nc.sync.dma_start(
                        zt[:mrows, :, :],
                        lv3[:, 128 * mc:128 * mc + mrows, :].transpose([1, 0, 2]))
                    nc.vector.tensor_tensor(v_sb[:mrows, mc, :, 0:D],
                                            v_sb[:mrows, mc, :, 0:D],
                                            zt[:mrows, :, :], ADD)

            # ---------------- main attention loop ----------------
            rec_dram = dp.tile([NCH, HEAD * NF], F32)
            with tc.tile_pool(name="stream", bufs=2) as sp:
                for ncx in range(NCH):
                    nsl = slice(NF * ncx, NF * ncx + NF)

                    xT_sb = sp.tile([128, 4, NF], F32R, tag="xTc")
                    nc.gpsimd.dma_start(xT_sb[:], xT_ch[:, :, nsl])

                    tq_sb = sp.tile([R, NF], F32R, tag="tq")
                    ptq = psA.tile([128, 512], F32, tag="psa", name=f"tq_{ncx}")
                    for K in range(4):
                        nc.tensor.matmul(ptq[:R, :NF], aq_sb[:, K, :],
                                         xT_sb[:, K, :],
                                         start=(K == 0), stop=(K == 3))
                    nc.scalar.copy(tq_sb[:], ptq[:R, :NF])

                    qT_sb = sp.tile([128, 4, NF], F32R, tag="qT")
                    for M in range(4):
                        pq = psA.tile([128, 512], F32, tag="psa",
                                      name=f"q_{ncx}_{M}")
                        for K in range(4):
                            nc.tensor.matmul(pq[:, :NF],
                                             wq_sb[:, K, 128 * M:128 * M + 128],
                                             xT_sb[:, K, :],
                                             start=(K == 0), stop=False)
                        nc.tensor.matmul(pq[:, :NF], bq_sb[:, 128 * M:128 * M + 128],
                                         tq_sb[:], start=False, stop=False)
                        nc.tensor.matmul(pq[:, :NF], bias_q[:, 128 * M:128 * M + 128],
                                         ones_r[:, :NF], start=False, stop=True)
                        nc.scalar.copy(qT_sb[:, M, :], pq[:, :NF])

                    av_ps = psAV.tile([D + 1, HEAD, NF], F32, tag="av", name=f"av_{ncx}")
                    for h in range(HEAD):
                        hb = 64 * (h % 2)
                        hc = h // 2
                        st_ps_t = psST.tile([128, 5 * NF], F32, tag="st", name=f"st_{ncx}_{h}")
                        for mc in range(5):
                            nc.tensor.matmul(
                                st_ps_t[:, NF * mc:NF * mc + NF],
                                kT_sb[hb:hb + 64, hc, 128 * mc:128 * mc + 128],
                                qT_sb[hb:hb + 64, hc, :],
                                start=True, stop=True)
                        est = sp.tile([128, 5 * NF], F32R, tag="est", bufs=2)
                        nc.scalar.activation(est[:], st_ps_t[:], Exp, scale=SM_SCALE)
                        for mc in range(5):
                            nc.tensor.matmul(av_ps[:, h, :], v_sb[:, mc, h, :],
                                             est[:, NF * mc:NF * mc + NF],
                                             start=(mc == 0), stop=(mc == 4))

                    # softmax denominators (row D of av_ps), recip + broadcast
                    srow = sp.tile([1, HEAD, NF], F32, tag="srow")
                    nc.vector.tensor_copy(srow[:], av_ps[D:D + 1, :, :])
                    rec_sb = sp.tile([1, HEAD, NF], F32, tag="rec")
                    nc.vector.reciprocal_approx_fast(rec_sb[:], srow[:])
                    nc.sync.dma_start(rec_dram[ncx, :][None, :],
                                      rec_sb[:].rearrange("p h n -> p (h n)"))
                    recb = sp.tile([128, HEAD, NF], F32, tag="recb")
                    nc.sync.dma_start(
                        recb[:],
                        rec_dram[ncx, :].rearrange("(h n) -> h n", h=HEAD)
                        [None, :, :].broadcast_to((128, HEAD, NF)))

                    outT_sb = sp.tile([128, 4, NF], F32R, tag="outT")
                    nc.vector.tensor_tensor(outT_sb[0:64, :, :],
                                            av_ps[0:D, 0::2, :],
                                            recb[0:64, 0::2, :], MULT)
                    nc.vector.tensor_tensor(outT_sb[64:128, :, :],
                                            av_ps[0:D, 1::2, :],
                                            recb[64:128, 1::2, :], MULT)

                    for Mn in range(NF // 128):
                        po = psA.tile([128, 512], F32, tag="psa",
                                      name=f"o_{ncx}_{Mn}")
                        for K in range(4):
                            nc.tensor.matmul(po[:],
                                             outT_sb[:, K, 128 * Mn:128 * Mn + 128],
                                             wp_sb[:, K, :],
                                             start=(K == 0), stop=False)
                        nc.tensor.matmul(po[:], ones_r[:, :128], bias_p[:],
                                         start=False, stop=True)
                        o_sb = sp.tile([128, C], F32, tag="osb")
                        nc.scalar.copy(o_sb[:], po[:])
                        nc.sync.dma_start(
                            out_d.ap()[NF * ncx + 128 * Mn:
                                       NF * ncx + 128 * Mn + 128, :],
                            o_sb[:])

    nc.compile()
    return nc


def host_prep(x, w_q, b_q, w_kv, b_kv, w_proj, b_proj, w_sr, b_sr,
              ln_g, ln_b, lora_A_q, lora_B_q, lora_A_v, lora_B_v):
    """Build the 8 per-core input dicts (all numpy fp32)."""
    f = np.float32
    w_k = w_kv[:C]
    w_v = w_kv[C:]
    w_k_eff = (w_k * ln_g[None, :]).astype(f)
    w_v_eff = (w_v * ln_g[None, :]).astype(f)
    b_k_eff = (b_kv[:C] + w_k @ ln_b).astype(f)
    b_v_eff = (b_kv[C:] + w_v @ ln_b).astype(f)
    A_v_eff = (lora_A_v * ln_g[None, :]).astype(f)
    avb = (lora_A_v @ ln_b).astype(f)
    B_q_s = (lora_B_q * SCALING).astype(f)
    B_v_s = (lora_B_v * SCALING).astype(f)

    w_flatT = np.ascontiguousarray(
        w_sr.transpose(2, 3, 1, 0).reshape(16 * C, C)).astype(f)

    shared = {
        "wqT": np.ascontiguousarray(w_q.T).astype(f),
        "wkT": np.ascontiguousarray(w_k_eff.T),
        "wvT": np.ascontiguousarray(w_v_eff.T),
        "wpT": np.ascontiguousarray(w_proj.T).astype(f),
        "aqT": np.ascontiguousarray(lora_A_q.T).astype(f),
        "bqT": np.ascontiguousarray(B_q_s.T),
        "avT": np.ascontiguousarray(A_v_eff.T),
        "bvT": np.ascontiguousarray(B_v_s.T),
        "b_q": b_q.reshape(1, C).astype(f),
        "b_k": b_k_eff.reshape(1, C),
        "b_v": b_v_eff.reshape(1, C),
        "b_sr4": (b_sr / 4.0).reshape(1, C).astype(f),
        "b_p": b_proj.reshape(1, C).astype(f),
        "avb": avb.reshape(1, R),
    }

    in_maps = []
    for core in range(N_CORES):
        b = core // 4
        g = core % 4
        xb = np.asarray(x[b], f)                      # (9216, 512)
        xT_c = np.ascontiguousarray(xb[NCHUNK * g:NCHUNK * (g + 1), :].T)
        patches = np.ascontiguousarray(
            xb.reshape(24, 4, 24, 4, C).transpose(1, 3, 4, 0, 2).reshape(16 * C, NKV))
        m = dict(shared)
        m["xT"] = xT_c
        m["patchT"] = np.ascontiguousarray(patches[CIN_SL * g:CIN_SL * (g + 1), :])
        m["wsrT"] = np.ascontiguousarray(w_flatT[CIN_SL * g:CIN_SL * (g + 1), :])
        in_maps.append(m)
    return in_maps


_NC_CACHE = {}


def kernel(x, w_q, b_q, w_kv, b_kv, w_proj, b_proj, w_sr, b_sr,
           ln_g, ln_b, lora_A_q, lora_B_q, lora_A_v, lora_B_v, H, W):
    assert int(H) == 96 and int(W) == 96
    args = [x, w_q, b_q, w_kv, b_kv, w_proj, b_proj, w_sr, b_sr,
            ln_g, ln_b, lora_A_q, lora_B_q, lora_A_v, lora_B_v]
    args = [np.asarray(a, np.float32) for a in args]
    in_maps = host_prep(*args)
    if "nc" not in _NC_CACHE:
        _NC_CACHE["nc"] = build_kernel()
    nc = _NC_CACHE["nc"]
    res = bass_utils.run_bass_kernel_spmd(
        nc, in_maps, core_ids=list(range(N_CORES)))
    out = np.empty((B, N, C), np.float32)
    for core in range(N_CORES):
        b, g = core // 4, core % 4
        out[b, NCHUNK * g:NCHUNK * (g + 1), :] = res.results[core]["out"]
    return out
